# revision 11
# baseline (speedup 1.0000x reference)
"""GNN message-passing kernel v4 for trn2 (8 cores).

h = relu(BN(s1 @ W_pre)); agg = segment_sum(h[src], dst);
out = relu((1-b)*support + b*support@W_op), support = 0.9*(h+agg) + 0.1*x_0.

Sharding: s1/x_0/dst sharded by node across 8 cores.
- P1: per-core z = s1_m @ W (bf16) chunk matmuls; column sums of z and z^2
  accumulated in PSUM via ones-lhsT matmuls; z stored in SBUF (bf16).
- AR: AllReduce the [1,512] partial sums -> global BN stats; broadcast the
  affine (a, b) rows.
- P3: h_m = relu(a*z + b) -> bounce to DRAM (bf16, chunk-major).
- AG: AllGather h shards -> full gather table [1024, 49*256] in local DRAM.
- C: dst tiles grouped into blocks of TB tiles; edges packed per
  (block, table-half) with no per-tile chunk padding (chunks may straddle
  tile boundaries -> one selection matmul per (chunk, tile) pair). One
  dma_gather per (block, half) with exact runtime count; -1 padding
  indices are skipped by HW. x0 (pre-scaled bf16, SBUF-resident) and the
  self term enter the PSUM accumulation via identity matmuls. Epilogue
  applies W_eff = I + scale*W_op so no extra adds are needed.
"""
import math
import numpy as np
import ml_dtypes

import concourse.bass as bass
import concourse.bacc as bacc
import concourse.mybir as mybir
from concourse.tile import TileContext

BF16 = mybir.dt.bfloat16
F32 = mybir.dt.float32
I16 = mybir.dt.int16

ALPHA = 0.1
LAMBDA = 0.5
BN_EPS = 1e-5
BETA_C = float(np.log(LAMBDA / 1.0 + 1.0))   # 0.405465
W_OP_SCALE = BETA_C / (1.0 - BETA_C)
OUT_SCALE = 1.0 - BETA_C
X0_SCALE = ALPHA / (1.0 - ALPHA)
TB = 4                                        # dst tiles per gather block


class Prob:
    def __init__(self, N, E, C, HID, n_cores):
        self.N, self.E, self.C, self.HID, self.n_cores = N, E, C, HID, n_cores
        assert C == 256 and HID == 256
        self.shard = N // n_cores                      # nodes per core
        assert self.shard * n_cores == N
        self.tiles = math.ceil(self.shard / 128)       # chunks per shard
        self.shard_pad = self.tiles * 128
        self.nb = math.ceil(self.tiles / TB)           # gather blocks
        self.half_rows = (n_cores // 2) * 128 * self.tiles
        assert self.half_rows < 32768


def host_prep(prob, s1, x_0, edge_index):
    """Per-core input maps + static schedule. All numpy."""
    p = prob
    N, E, M = p.N, p.E, p.n_cores
    s1 = np.asarray(s1, dtype=np.float32)
    x_0 = np.asarray(x_0, dtype=np.float32)
    src = np.asarray(edge_index[0], dtype=np.int64)
    dst = np.asarray(edge_index[1], dtype=np.int64)

    core = dst // p.shard
    rel = dst - core * p.shard
    t_of = rel >> 7                     # dst tile within shard
    prel = rel & 127                    # dst row within tile
    b_of = t_of // TB                   # gather block
    sr = src // p.shard
    sl = src - sr * p.shard
    sc = sl >> 7
    sp = sl & 127
    srow = (sr * 128 + sp) * p.tiles + sc
    half = (srow >= p.half_rows).astype(np.int64)
    srow = srow - half * p.half_rows

    NB = p.nb
    ngroups = M * NB * 2
    group = (core * NB + b_of) * 2 + half
    order = np.lexsort((srow, t_of, group))
    g_s = group[order]
    t_s = t_of[order]
    prel_s = prel[order]
    srow_s = srow[order]

    gcnt = np.bincount(group, minlength=ngroups)           # [M*NB*2]
    gstart = np.zeros(ngroups + 1, np.int64)
    np.cumsum(gcnt, out=gstart[1:])
    pos = np.arange(E, dtype=np.int64) - gstart[g_s]       # pos within group
    chunk = pos >> 7
    slot = pos & 127

    cnt_mbh = gcnt.reshape(M, NB, 2)
    num_reg = cnt_mbh.max(axis=0)                          # [NB, 2]
    K = np.maximum((num_reg + 127) // 128, 1)              # chunks per call
    off = np.zeros((NB, 2), np.int64)                      # chunk offset
    run = 0
    for b in range(NB):
        for h in (0, 1):
            off[b, h] = run
            run += K[b, h]
    ktot = run

    # ---- static (b, h, chunk, tile) pair schedule (union over cores) ----
    bh_s = g_s % (NB * 2)                                  # (b*2 + h)
    Kmax = int(K.max())
    pair_key = (bh_s * (Kmax + 1) + chunk) * 64 + (t_s - (bh_s // 2) * TB)
    upairs, inv = np.unique(pair_key, return_inverse=True)
    npairs = len(upairs)
    u_bh = upairs // ((Kmax + 1) * 64)
    u_rest = upairs % ((Kmax + 1) * 64)
    u_chunk = u_rest // 64
    u_tin = u_rest % 64
    u_b = u_bh // 2
    u_h = u_bh % 2
    u_t = u_b * TB + u_tin
    # pair order: already sorted by (b, h, chunk, tile) via key  ✓
    pairs = [[] for _ in range(NB)]     # per block: list of (h, c, t, j)
    for j in range(npairs):
        pairs[int(u_b[j])].append((int(u_h[j]), int(u_chunk[j]), int(u_t[j]), j))

    # ---- per-core flat layouts ----
    edge_core = g_s // (NB * 2)
    flatpos = (off[u_b[inv], u_h[inv]] * 128 + pos)        # idx slot per edge
    idx_flat = np.zeros((M, ktot * 128), np.int16)
    idx_flat[edge_core, flatpos] = srow_s.astype(np.int16)
    drel_flat = np.full((M, 128, npairs), 200.0, np.float32)
    drel_flat[edge_core, slot, inv] = prel_s.astype(np.float32)

    idx_lay = idx_flat.reshape(M, ktot * 8, 16).transpose(0, 2, 1)
    idx_lay = np.tile(idx_lay, (1, 8, 1))                  # [M, 128, ktot*8]

    # per-core s1 shard, transposed+padded: [C, shard_pad] bf16
    s1Ts = np.zeros((M, p.C, p.shard_pad), np.float32)
    for m in range(M):
        s1Ts[m, :, :p.shard] = s1[m * p.shard:(m + 1) * p.shard].T
    s1Ts = s1Ts.astype(ml_dtypes.bfloat16)

    # x0 pre-scaled, bf16, chunk-major [128, tiles*HID]
    x0s = np.zeros((M, p.shard_pad, p.HID), np.float32)
    for m in range(M):
        x0s[m, :p.shard] = X0_SCALE * x_0[m * p.shard:(m + 1) * p.shard]
    x0s = x0s.reshape(M, p.tiles, 128, p.HID).transpose(0, 2, 1, 3).reshape(M, 128, -1)
    x0s = np.ascontiguousarray(x0s).astype(ml_dtypes.bfloat16)

    iota = np.broadcast_to(np.arange(128, dtype=np.float32), (128, 128)).astype(ml_dtypes.bfloat16).copy()
    ident = np.eye(128, dtype=np.float32)
    ones1 = np.ones((1, 128), np.float32)
    onesc = np.ones((128, 1), np.float32)

    meta = dict(K=K, off=off, ktot=ktot, num_reg=num_reg, pairs=pairs,
                npairs=npairs, Kmax=Kmax)
    shared = dict(iota=iota, ident=ident, ones1=ones1, onesc=onesc)
    in_maps = []
    for m in range(M):
        d = dict(shared)
        d["s1T"] = np.ascontiguousarray(s1Ts[m])
        d["idxall"] = np.ascontiguousarray(idx_lay[m])
        d["drel"] = np.ascontiguousarray(drel_flat[m])
        d["x0s"] = x0s[m]
        in_maps.append(d)
    return in_maps, meta


def build_kernel(prob, meta, W_pre, gamma, beta_bn, W_op, nloop=1, nq=1,
                 phases="13C", coll=True):
    p = prob
    K, off, ktot = meta["K"], meta["off"], meta["ktot"]
    num_reg, pairs, npairs = meta["num_reg"], meta["pairs"], meta["npairs"]
    Kmax = meta["Kmax"]
    C, HID = p.C, p.HID
    M = p.n_cores
    cpr = p.tiles
    NB = p.nb
    nc = bacc.Bacc("TRN2", target_bir_lowering=False, debug=False,
                   num_devices=M, num_swdge_queues=nq)
    t_s1T = nc.dram_tensor("s1T", [C, p.shard_pad], BF16, kind="ExternalInput")
    t_wpre = nc.dram_tensor("wpre", [C, HID], F32, kind="ExternalInput")
    t_gamma = nc.dram_tensor("gamma", [1, HID], F32, kind="ExternalInput")
    t_beta = nc.dram_tensor("beta", [1, HID], F32, kind="ExternalInput")
    t_wop = nc.dram_tensor("wop", [HID, HID], F32, kind="ExternalInput")
    t_x0 = nc.dram_tensor("x0s", [128, cpr * HID], BF16, kind="ExternalInput")
    t_idx = nc.dram_tensor("idxall", [128, ktot * 8], I16, kind="ExternalInput")
    t_drel = nc.dram_tensor("drel", [128, npairs], F32, kind="ExternalInput")
    t_iota = nc.dram_tensor("iota", [128, 128], BF16, kind="ExternalInput")
    t_ident = nc.dram_tensor("ident", [128, 128], F32, kind="ExternalInput")
    t_ones1 = nc.dram_tensor("ones1", [1, 128], F32, kind="ExternalInput")
    t_onesc = nc.dram_tensor("onesc", [128, 1], F32, kind="ExternalInput")
    t_out = nc.dram_tensor("out", [128, cpr * HID], F32, kind="ExternalOutput")
    # collective buffers
    t_arin = nc.dram_tensor("arin", [1, 2 * HID], F32)
    t_arout = nc.dram_tensor("arout", [1, 2 * HID], F32)
    t_hb = nc.dram_tensor("hb", [128, cpr * HID], BF16)
    t_ag = nc.dram_tensor("agh", [M * 128, cpr * HID], BF16)
    groups = [list(range(M))]

    def loop(tc, body, active):
        if active and nloop > 1:
            with tc.For_i(0, nloop, 1):
                body()
        else:
            body()

    tc1 = TileContext(nc)
    with tc1 as tc:
        with (tc.tile_pool(name="const", bufs=1) as cpool,
              tc.tile_pool(name="work", bufs=5) as wpool,
              tc.tile_pool(name="hout", bufs=4) as hpool,
              tc.tile_pool(name="psZ", bufs=4, space="PSUM") as psZ,
              tc.tile_pool(name="psB", bufs=1, space="PSUM") as psB,
              tc.tile_pool(name="psS", bufs=1, space="PSUM") as psS,
              tc.tile_pool(name="small", bufs=1) as smpool):
            # ---- constants ----
            s1sb = []
            for r in range(2):
                t = cpool.tile([128, p.shard_pad], BF16, tag=f"s1{r}")
                nc.sync.dma_start(out=t[:], in_=t_s1T[r * 128:(r + 1) * 128, :])
                s1sb.append(t)
            w_f32, w_bf = [], []
            for r in range(2):
                w = cpool.tile([128, HID], F32, tag=f"wf{r}")
                nc.sync.dma_start(out=w[:], in_=t_wpre[r * 128:(r + 1) * 128, :])
                w_f32.append(w)
                wb = cpool.tile([128, HID], BF16, tag=f"wb{r}")
                nc.vector.tensor_copy(out=wb[:], in_=w[:])
                w_bf.append(wb)
            gamma_sb = cpool.tile([1, HID], F32, tag="gm")
            nc.sync.dma_start(out=gamma_sb[:], in_=t_gamma[:])
            beta_sb = cpool.tile([1, HID], F32, tag="bt")
            nc.sync.dma_start(out=beta_sb[:], in_=t_beta[:])
            ones1_sb = cpool.tile([1, 128], F32, tag="on")
            nc.sync.dma_start(out=ones1_sb[:], in_=t_ones1[:])
            onesc_sb = cpool.tile([128, 1], F32, tag="onc")
            nc.sync.dma_start(out=onesc_sb[:], in_=t_onesc[:])
            onesc_bf = cpool.tile([128, 1], BF16, tag="oncb")
            nc.vector.tensor_copy(out=onesc_bf[:], in_=onesc_sb[:])

            # ---- P1: z -> SBUF store; squares + column-sum matmuls ----
            sums_z = psS.tile([1, HID], F32, tag="sz", name="sums_z")
            sums_q = psS.tile([1, HID], F32, tag="sq", name="sums_q")
            zst = cpool.tile([128, cpr * HID], BF16, tag="zst")
            zqt = cpool.tile([128, cpr * HID], BF16, tag="zqt")
            def phase1():
                for j in range(cpr):
                    zc = psZ.tile([128, HID], F32, tag="zc")
                    co = j * 128
                    nc.tensor.matmul(zc[:], lhsT=s1sb[0][:, co:co + 128],
                                     rhs=w_bf[0][:], start=True, stop=False)
                    nc.tensor.matmul(zc[:], lhsT=s1sb[1][:, co:co + 128],
                                     rhs=w_bf[1][:], start=False, stop=True)
                    if j % 2 == 0:
                        nc.vector.tensor_copy(out=zst[:, j * HID:(j + 1) * HID],
                                              in_=zc[:])
                    else:
                        nc.scalar.activation(out=zst[:, j * HID:(j + 1) * HID],
                                             in_=zc[:],
                                             func=mybir.ActivationFunctionType.Copy,
                                             bias=0.0, scale=1.0)
                for j in range(cpr):
                    nc.scalar.activation(out=zqt[:, j * HID:(j + 1) * HID],
                                         in_=zst[:, j * HID:(j + 1) * HID],
                                         func=mybir.ActivationFunctionType.Square,
                                         bias=0.0, scale=1.0)
                for j in range(cpr):
                    nc.tensor.matmul(sums_z[:], lhsT=onesc_bf[:],
                                     rhs=zst[:, j * HID:(j + 1) * HID],
                                     start=(j == 0), stop=(j == cpr - 1))
                    nc.tensor.matmul(sums_q[:], lhsT=onesc_bf[:],
                                     rhs=zqt[:, j * HID:(j + 1) * HID],
                                     start=(j == 0), stop=(j == cpr - 1))
            loop(tc, phase1, "1" in phases)

            # ---- AR: allreduce stats ----
            sums_sb = smpool.tile([1, 2 * HID], F32, tag="ssb")
            nc.vector.tensor_copy(out=sums_sb[:, :HID], in_=sums_z[:])
            nc.vector.tensor_copy(out=sums_sb[:, HID:], in_=sums_q[:])
            nc.sync.dma_start(out=t_arin[:], in_=sums_sb[:])
            def phase_ar():
                nc.gpsimd.collective_compute(
                    "AllReduce", mybir.AluOpType.add, replica_groups=groups,
                    ins=[t_arin[:]], outs=[t_arout[:]])
            if coll:
                loop(tc, phase_ar, "r" in phases)
            stats_sb = smpool.tile([1, 2 * HID], F32, tag="stats")
            nc.sync.dma_start(out=stats_sb[:],
                              in_=(t_arout[:] if coll else t_arin[:]))

            # ---- stats finalize ----
            invn = 1.0 / p.N
            mu = smpool.tile([1, HID], F32, tag="mu")
            nc.vector.tensor_scalar(out=mu[:], in0=stats_sb[:, :HID], scalar1=invn,
                                    scalar2=None, op0=mybir.AluOpType.mult)
            var = smpool.tile([1, HID], F32, tag="var")
            nc.vector.tensor_scalar(out=var[:], in0=stats_sb[:, HID:], scalar1=invn,
                                    scalar2=None, op0=mybir.AluOpType.mult)
            musq = smpool.tile([1, HID], F32, tag="musq")
            nc.vector.tensor_tensor(out=musq[:], in0=mu[:], in1=mu[:],
                                    op=mybir.AluOpType.mult)
            nc.vector.tensor_tensor(out=var[:], in0=var[:], in1=musq[:],
                                    op=mybir.AluOpType.subtract)
            nc.vector.tensor_scalar(out=var[:], in0=var[:], scalar1=BN_EPS,
                                    scalar2=None, op0=mybir.AluOpType.add)
            sq = smpool.tile([1, HID], F32, tag="sqr")
            nc.scalar.activation(out=sq[:], in_=var[:],
                                 func=mybir.ActivationFunctionType.Sqrt,
                                 bias=0.0, scale=1.0)
            rs = smpool.tile([1, HID], F32, tag="rs")
            nc.vector.reciprocal(out=rs[:], in_=sq[:])
            a_vec = smpool.tile([1, HID], F32, tag="av")
            nc.vector.tensor_tensor(out=a_vec[:], in0=rs[:], in1=gamma_sb[:],
                                    op=mybir.AluOpType.mult)
            b_vec = smpool.tile([1, HID], F32, tag="bv")
            nc.vector.tensor_tensor(out=b_vec[:], in0=mu[:], in1=a_vec[:],
                                    op=mybir.AluOpType.mult)
            nc.vector.tensor_tensor(out=b_vec[:], in0=beta_sb[:], in1=b_vec[:],
                                    op=mybir.AluOpType.subtract)
            ps_ab = psB.tile([128, HID], F32, tag="zab", name="ps_ab")
            nc.tensor.matmul(ps_ab[:], lhsT=ones1_sb[:], rhs=a_vec[:],
                             start=True, stop=True)
            a_bc = cpool.tile([128, HID], F32, tag="abc2")
            nc.vector.tensor_copy(out=a_bc[:], in_=ps_ab[:])
            # fold the BN scale into the weights: W' = W * a  (bf16)
            w_sc = []
            for r in range(2):
                wpr = cpool.tile([128, HID], BF16, tag=f"wpr{r}")
                nc.vector.tensor_tensor(out=wpr[:], in0=w_f32[r][:], in1=a_bc[:],
                                        op=mybir.AluOpType.mult)
                w_sc.append(wpr)

            # ---- P3: h = relu(s1 @ W' + b) (PE + Act only) ----
            HSPAN = 7
            def phase3():
                for j in range(cpr):
                    hc = psZ.tile([128, HID], F32, tag="hc")
                    co = j * 128
                    nc.tensor.matmul(hc[:], lhsT=s1sb[0][:, co:co + 128],
                                     rhs=w_sc[0][:], start=True, stop=False)
                    nc.tensor.matmul(hc[:], lhsT=s1sb[1][:, co:co + 128],
                                     rhs=w_sc[1][:], start=False, stop=False)
                    nc.tensor.matmul(hc[:], lhsT=ones1_sb[:], rhs=b_vec[:],
                                     start=False, stop=True)
                    hs = j // HSPAN
                    ho = j % HSPAN
                    he = min(cpr, (hs + 1) * HSPAN) - hs * HSPAN
                    if ho == 0:
                        hsp = hpool.tile([128, HSPAN * HID], BF16, tag="hsp",
                                         name=f"hsp_{hs % 4}")
                        phase3.hsp = hsp
                    hsp = phase3.hsp
                    hj = hsp[:, ho * HID:(ho + 1) * HID]
                    if j % 2 == 0:
                        nc.scalar.activation(out=hj, in_=hc[:],
                                             func=mybir.ActivationFunctionType.Relu,
                                             bias=0.0, scale=1.0)
                    else:
                        nc.vector.tensor_scalar(out=hj, in0=hc[:], scalar1=0.0,
                                                scalar2=None,
                                                op0=mybir.AluOpType.max)
                    if ho == he - 1:
                        nc.sync.dma_start(
                            out=t_hb[:, hs * HSPAN * HID:(hs * HSPAN + he) * HID],
                            in_=hsp[:, :he * HID])
            loop(tc, phase3, "3" in phases)

            # ---- AG: allgather h shards ----
            def phase_ag():
                nc.gpsimd.collective_compute(
                    "AllGather", mybir.AluOpType.bypass, replica_groups=groups,
                    ins=[t_hb[:]], outs=[t_ag[:]])
            if coll:
                loop(tc, phase_ag, "G" in phases)

        # ---------------- phase C: aggregate + output ----------------
        with (tc.tile_pool(name="c2", bufs=1) as cpool,
              tc.tile_pool(name="gat", bufs=1) as gpool,
              tc.tile_pool(name="sel", bufs=16) as selp,
              tc.tile_pool(name="epi", bufs=3) as epool,
              tc.tile_pool(name="osp", bufs=2) as opool,
              tc.tile_pool(name="psG", bufs=4, space="PSUM") as psG,
              tc.tile_pool(name="psT", bufs=2, space="PSUM") as psT,
              tc.tile_pool(name="psO", bufs=2, space="PSUM") as psO):
            idx_sb = cpool.tile([128, ktot * 8], I16, tag="idx")
            nc.sync.dma_start(out=idx_sb[:], in_=t_idx[:])
            drel_sb = cpool.tile([128, npairs], F32, tag="dr")
            nc.sync.dma_start(out=drel_sb[:], in_=t_drel[:])
            iota_sb = cpool.tile([128, 128], BF16, tag="io")
            nc.sync.dma_start(out=iota_sb[:], in_=t_iota[:])
            ident_sb = cpool.tile([128, 128], F32, tag="idn")
            nc.sync.dma_start(out=ident_sb[:], in_=t_ident[:])
            ident_bf = cpool.tile([128, 128], BF16, tag="idnb")
            nc.vector.tensor_copy(out=ident_bf[:], in_=ident_sb[:])
            hsh = cpool.tile([128, cpr * HID], BF16, tag="hsh")
            nc.sync.dma_start(out=hsh[:], in_=t_hb[:])
            x0sb = cpool.tile([128, cpr * HID], BF16, tag="x0sb")
            nc.sync.dma_start(out=x0sb[:], in_=t_x0[:])
            # W_eff = I + W_OP_SCALE * W_op  (bf16, 2 chunks)
            weff = []
            for r in range(2):
                w = cpool.tile([128, HID], F32, tag=f"wo2{r}")
                nc.sync.dma_start(out=w[:], in_=t_wop[r * 128:(r + 1) * 128, :])
                wsc = cpool.tile([128, HID], F32, tag=f"wsc{r}")
                nc.vector.tensor_scalar(out=wsc[:], in0=w[:], scalar1=W_OP_SCALE,
                                        scalar2=None, op0=mybir.AluOpType.mult)
                nc.vector.tensor_tensor(
                    out=wsc[:, r * 128:(r + 1) * 128],
                    in0=wsc[:, r * 128:(r + 1) * 128], in1=ident_sb[:],
                    op=mybir.AluOpType.add)
                wb = cpool.tile([128, HID], BF16, tag=f"wo2b{r}")
                nc.vector.tensor_copy(out=wb[:], in_=wsc[:])
                weff.append(wb)

            half_parts = (M // 2) * 128
            tbl = [t_ag[0:half_parts, :], t_ag[half_parts:2 * half_parts, :]]

            # gather buffers (uniform size), memset once for -1-skip safety
            for h in (0, 1):
                for par in (0, 1):
                    g = gpool.tile([128, Kmax * HID], BF16, name=f"g{h}_{par}")
                    nc.vector.memset(g[:], 0.0)
            qn = [0]

            # per-tile matmul counts for stop flags
            nmm = [2] * cpr
            for b in range(NB):
                for (h, c, t, j) in pairs[b]:
                    nmm[t] += 1

            OSP = TB
            def phaseC():
                for b in range(NB):
                    t0 = b * TB
                    t1 = min(cpr, t0 + TB)
                    par = b % 2
                    gb = {}
                    for h in (0, 1):
                        kk = int(K[b, h])
                        g = gpool.tile([128, Kmax * HID], BF16, name=f"g{h}_{par}")
                        o8 = int(off[b, h]) * 8
                        nc.gpsimd.dma_gather(
                            out_ap=g[:, :kk * HID].rearrange("p (c d) -> p c d", d=HID),
                            in_ap=tbl[h].rearrange("q (c d) -> (q c) d", d=HID),
                            idxs_ap=idx_sb[:, o8:o8 + kk * 8],
                            num_idxs=kk * 128, num_idxs_reg=kk * 128,
                            elem_size=HID, single_packet=False,
                            queue_num=qn[0] % nq)
                        qn[0] += 1
                        gb[h] = g
                    aggs = {}
                    done = {}
                    for t in range(t0, t1):
                        agg = psG.tile([128, HID], F32, tag="agg",
                                       name=f"agg{t % 4}")
                        nc.tensor.matmul(agg[:], lhsT=ident_bf[:],
                                         rhs=hsh[:, t * HID:(t + 1) * HID],
                                         start=True, stop=False)
                        nc.tensor.matmul(agg[:], lhsT=ident_bf[:],
                                         rhs=x0sb[:, t * HID:(t + 1) * HID],
                                         start=False, stop=(nmm[t] == 2))
                        aggs[t] = agg
                        done[t] = 2
                    for (h, c, t, j) in pairs[b]:
                        S = selp.tile([128, 128], BF16)
                        nc.vector.tensor_scalar(
                            out=S[:], in0=iota_sb[:],
                            scalar1=drel_sb[:, j:j + 1], scalar2=None,
                            op0=mybir.AluOpType.is_equal)
                        done[t] += 1
                        nc.tensor.matmul(aggs[t][:], lhsT=S[:],
                                         rhs=gb[h][:, c * HID:(c + 1) * HID],
                                         start=False, stop=(done[t] == nmm[t]))
                    # epilogue for the block
                    outsp = opool.tile([128, OSP * HID], F32, tag="outsp")
                    for t in range(t0, t1):
                        eo = t - t0
                        sup = epool.tile([128, HID], F32, tag="sup")
                        nc.scalar.activation(out=sup[:], in_=aggs[t][:],
                                             func=mybir.ActivationFunctionType.Copy,
                                             bias=0.0, scale=(1.0 - ALPHA))
                        trp = psT.tile([128, HID], F32, tag="tr")
                        for r in range(2):
                            nc.tensor.transpose(
                                out=trp[:, r * 128:(r + 1) * 128],
                                in_=sup[:, r * 128:(r + 1) * 128],
                                identity=ident_sb[:])
                        supT = epool.tile([128, HID], BF16, tag="supT")
                        if t % 2 == 0:
                            nc.vector.tensor_copy(out=supT[:], in_=trp[:])
                        else:
                            nc.scalar.activation(
                                out=supT[:], in_=trp[:],
                                func=mybir.ActivationFunctionType.Copy,
                                bias=0.0, scale=1.0)
                        ops = psO.tile([128, HID], F32, tag="o")
                        nc.tensor.matmul(ops[:], lhsT=supT[:, :128], rhs=weff[0][:],
                                         start=True, stop=False)
                        nc.tensor.matmul(ops[:], lhsT=supT[:, 128:], rhs=weff[1][:],
                                         start=False, stop=True)
                        nc.scalar.activation(out=outsp[:, eo * HID:(eo + 1) * HID],
                                             in_=ops[:],
                                             func=mybir.ActivationFunctionType.Relu,
                                             bias=0.0, scale=OUT_SCALE)
                    nc.sync.dma_start(out=t_out[:, t0 * HID:t1 * HID],
                                      in_=outsp[:, :(t1 - t0) * HID])
            loop(tc, phaseC, "C" in phases)

    nc.compile()
    return nc


def make_weight_inputs(prob, W_pre, gamma, beta_bn, W_op):
    return dict(
        wpre=np.asarray(W_pre, np.float32),
        gamma=np.asarray(gamma, np.float32).reshape(1, -1),
        beta=np.asarray(beta_bn, np.float32).reshape(1, -1),
        wop=np.asarray(W_op, np.float32),
    )


def unpack_out(prob, arr):
    return arr.reshape(128, prob.tiles, prob.HID).transpose(1, 0, 2).reshape(
        prob.shard_pad, prob.HID)


# ======================================================================
# Self-contained execution via PJRT (axon)
# ======================================================================
import jax
from jax.sharding import Mesh, PartitionSpec, NamedSharding
from jax.experimental.shard_map import shard_map
from concourse.bass2jax import _bass_exec_p, install_neuronx_cc_hook, partition_id_tensor


def _build_exec(nc, n_cores):
    install_neuronx_cc_hook()
    partition_name = nc.partition_id_tensor.name if nc.partition_id_tensor else None
    in_names, out_names, out_avals, zero_outs = [], [], [], []
    for alloc in nc.m.functions[0].allocations:
        if not isinstance(alloc, mybir.MemoryLocationSet):
            continue
        name = alloc.memorylocations[0].name
        if alloc.kind == "ExternalInput":
            if name != partition_name:
                in_names.append(name)
        elif alloc.kind == "ExternalOutput":
            shape = tuple(alloc.tensor_shape)
            dtype = mybir.dt.np(alloc.dtype)
            out_names.append(name)
            out_avals.append(jax.core.ShapedArray(shape, dtype))
            zero_outs.append(np.zeros(shape, dtype))
    n_params = len(in_names)
    n_outs = len(out_avals)
    all_in_names = list(in_names) + list(out_names)
    if partition_name is not None:
        all_in_names.append(partition_name)

    def _body(*args):
        operands = list(args)
        if partition_name is not None:
            operands.append(partition_id_tensor())
        outs = _bass_exec_p.bind(
            *operands, out_avals=tuple(out_avals), in_names=tuple(all_in_names),
            out_names=tuple(out_names), lowering_input_output_aliases=(),
            sim_require_finite=True, sim_require_nnan=True, nc=nc)
        return tuple(outs)

    devices = jax.devices()[:n_cores]
    mesh = Mesh(np.asarray(devices), ("core",))
    in_specs = (PartitionSpec("core"),) * (n_params + n_outs)
    out_specs = (PartitionSpec("core"),) * n_outs
    donate = tuple(range(n_params, n_params + n_outs))
    fn = jax.jit(shard_map(_body, mesh=mesh, in_specs=in_specs,
                           out_specs=out_specs, check_rep=False),
                 donate_argnums=donate, keep_unused=True)
    return dict(fn=fn, in_names=in_names, out_names=out_names,
                out_avals=out_avals, zero_outs=zero_outs, mesh=mesh,
                n_cores=n_cores)


def _place_inputs(ex, in_maps):
    sh = NamedSharding(ex["mesh"], PartitionSpec("core"))
    n_cores = ex["n_cores"]
    return [jax.device_put(
        np.concatenate([np.asarray(in_maps[c][name]) for c in range(n_cores)], axis=0), sh)
        for name in ex["in_names"]]


def _run(ex, dev_in):
    sh = NamedSharding(ex["mesh"], PartitionSpec("core"))
    n_cores = ex["n_cores"]
    zs = [jax.device_put(np.zeros((n_cores * z.shape[0], *z.shape[1:]), z.dtype), sh)
          for z in ex["zero_outs"]]
    outs = jax.block_until_ready(ex["fn"](*dev_in, *zs))
    return [
        {name: np.asarray(outs[i]).reshape(n_cores, *ex["out_avals"][i].shape)[c]
         for i, name in enumerate(ex["out_names"])}
        for c in range(n_cores)
    ]


_CACHE = {}


def _get_compiled(prob, meta, W_pre, gamma, beta_bn, W_op, key):
    if key not in _CACHE:
        nc = build_kernel(prob, meta, W_pre, gamma, beta_bn, W_op, nloop=1)
        _CACHE[key] = _build_exec(nc, prob.n_cores)
    return _CACHE[key]


def kernel(s0=None, s1=None, x_0=None, W_pre=None, gamma=None, beta_bn=None,
           W_op=None, edge_index=None, drop_prob=None, training=None, **_ignored):
    s1 = np.asarray(s1, np.float32)
    x_0 = np.asarray(x_0, np.float32)
    W_pre = np.asarray(W_pre, np.float32)
    gamma = np.asarray(gamma, np.float32)
    beta_bn = np.asarray(beta_bn, np.float32)
    W_op = np.asarray(W_op, np.float32)
    edge_index = np.asarray(edge_index)
    N, C = s1.shape
    HID = W_pre.shape[1]
    E = edge_index.shape[1]
    prob = Prob(N, E, C, HID, n_cores=8)
    in_maps, meta = host_prep(prob, s1, x_0, edge_index)
    key = (N, E, C, HID, int(np.int64(edge_index[:, ::97]).sum()), meta["ktot"])
    ex = _get_compiled(prob, meta, W_pre, gamma, beta_bn, W_op, key)
    wins = make_weight_inputs(prob, W_pre, gamma, beta_bn, W_op)
    full_maps = [{**m, **wins} for m in in_maps]
    dev_in = _place_inputs(ex, full_maps)
    res = _run(ex, dev_in)
    out = np.concatenate(
        [unpack_out(prob, res[m]["out"])[:prob.shard] for m in range(prob.n_cores)],
        axis=0)
    return np.ascontiguousarray(out[:N]).astype(np.float32)


# revision 12
# speedup vs baseline: 1.0911x; 1.0911x over previous
"""GNN message-passing kernel v4 for trn2 (8 cores).

h = relu(BN(s1 @ W_pre)); agg = segment_sum(h[src], dst);
out = relu((1-b)*support + b*support@W_op), support = 0.9*(h+agg) + 0.1*x_0.

Sharding: s1/x_0/dst sharded by node across 8 cores.
- P1: per-core z = s1_m @ W (bf16) chunk matmuls; column sums of z and z^2
  accumulated in PSUM via ones-lhsT matmuls; z stored in SBUF (bf16).
- AR: AllReduce the [1,512] partial sums -> global BN stats; broadcast the
  affine (a, b) rows.
- P3: h_m = relu(a*z + b) -> bounce to DRAM (bf16, chunk-major).
- AG: AllGather h shards -> full gather table [1024, 49*256] in local DRAM.
- C: dst tiles grouped into blocks of TB tiles; edges packed per
  (block, table-half) with no per-tile chunk padding (chunks may straddle
  tile boundaries -> one selection matmul per (chunk, tile) pair). One
  dma_gather per (block, half) with exact runtime count; -1 padding
  indices are skipped by HW. x0 (pre-scaled bf16, SBUF-resident) and the
  self term enter the PSUM accumulation via identity matmuls. Epilogue
  applies W_eff = I + scale*W_op so no extra adds are needed.
"""
import math
import numpy as np
import ml_dtypes

import concourse.bass as bass
import concourse.bacc as bacc
import concourse.mybir as mybir
from concourse.tile import TileContext

BF16 = mybir.dt.bfloat16
F32 = mybir.dt.float32
I16 = mybir.dt.int16

ALPHA = 0.1
LAMBDA = 0.5
BN_EPS = 1e-5
BETA_C = float(np.log(LAMBDA / 1.0 + 1.0))   # 0.405465
W_OP_SCALE = BETA_C / (1.0 - BETA_C)
OUT_SCALE = 1.0 - BETA_C
X0_SCALE = ALPHA / (1.0 - ALPHA)
TB = 4                                        # dst tiles per gather block


class Prob:
    def __init__(self, N, E, C, HID, n_cores):
        self.N, self.E, self.C, self.HID, self.n_cores = N, E, C, HID, n_cores
        assert C == 256 and HID == 256
        self.shard = N // n_cores                      # nodes per core
        assert self.shard * n_cores == N
        self.tiles = math.ceil(self.shard / 128)       # chunks per shard
        self.shard_pad = self.tiles * 128
        self.nb = math.ceil(self.tiles / TB)           # gather blocks
        self.half_rows = (n_cores // 2) * 128 * self.tiles
        assert self.half_rows < 32768


def host_prep(prob, s1, x_0, edge_index):
    """Per-core input maps + static schedule. All numpy."""
    p = prob
    N, E, M = p.N, p.E, p.n_cores
    s1 = np.asarray(s1, dtype=np.float32)
    x_0 = np.asarray(x_0, dtype=np.float32)
    src = np.asarray(edge_index[0], dtype=np.int64)
    dst = np.asarray(edge_index[1], dtype=np.int64)

    core = dst // p.shard
    rel = dst - core * p.shard
    t_of = rel >> 7                     # dst tile within shard
    prel = rel & 127                    # dst row within tile
    b_of = t_of // TB                   # gather block
    sr = src // p.shard
    sl = src - sr * p.shard
    sc = sl >> 7
    sp = sl & 127
    srow = (sr * 128 + sp) * p.tiles + sc
    half = (srow >= p.half_rows).astype(np.int64)
    srow = srow - half * p.half_rows

    NB = p.nb
    ngroups = M * NB * 2
    group = (core * NB + b_of) * 2 + half
    order = np.lexsort((srow, t_of, group))
    g_s = group[order]
    t_s = t_of[order]
    prel_s = prel[order]
    srow_s = srow[order]

    gcnt = np.bincount(group, minlength=ngroups)           # [M*NB*2]
    gstart = np.zeros(ngroups + 1, np.int64)
    np.cumsum(gcnt, out=gstart[1:])
    pos = np.arange(E, dtype=np.int64) - gstart[g_s]       # pos within group
    chunk = pos >> 7
    slot = pos & 127

    cnt_mbh = gcnt.reshape(M, NB, 2)
    num_reg = cnt_mbh.max(axis=0)                          # [NB, 2]
    K = np.maximum((num_reg + 127) // 128, 1)              # chunks per call
    off = np.zeros((NB, 2), np.int64)                      # chunk offset
    run = 0
    for b in range(NB):
        for h in (0, 1):
            off[b, h] = run
            run += K[b, h]
    ktot = run

    # ---- static (b, h, chunk, tile) pair schedule (union over cores) ----
    bh_s = g_s % (NB * 2)                                  # (b*2 + h)
    Kmax = int(K.max())
    pair_key = (bh_s * (Kmax + 1) + chunk) * 64 + (t_s - (bh_s // 2) * TB)
    upairs, inv = np.unique(pair_key, return_inverse=True)
    npairs = len(upairs)
    u_bh = upairs // ((Kmax + 1) * 64)
    u_rest = upairs % ((Kmax + 1) * 64)
    u_chunk = u_rest // 64
    u_tin = u_rest % 64
    u_b = u_bh // 2
    u_h = u_bh % 2
    u_t = u_b * TB + u_tin
    # pair order: already sorted by (b, h, chunk, tile) via key  ✓
    pairs = [[] for _ in range(NB)]     # per block: list of (h, c, t, j)
    for j in range(npairs):
        pairs[int(u_b[j])].append((int(u_h[j]), int(u_chunk[j]), int(u_t[j]), j))

    # ---- per-core flat layouts ----
    edge_core = g_s // (NB * 2)
    flatpos = (off[u_b[inv], u_h[inv]] * 128 + pos)        # idx slot per edge
    idx_flat = np.zeros((M, ktot * 128), np.int16)
    idx_flat[edge_core, flatpos] = srow_s.astype(np.int16)
    drel_flat = np.full((M, 128, npairs), 200.0, np.float32)
    drel_flat[edge_core, slot, inv] = prel_s.astype(np.float32)

    idx_lay = idx_flat.reshape(M, ktot * 8, 16).transpose(0, 2, 1)
    idx_lay = np.tile(idx_lay, (1, 8, 1))                  # [M, 128, ktot*8]

    # per-core s1 shard, transposed+padded: [C, shard_pad] bf16
    s1Ts = np.zeros((M, p.C, p.shard_pad), np.float32)
    for m in range(M):
        s1Ts[m, :, :p.shard] = s1[m * p.shard:(m + 1) * p.shard].T
    s1Ts = s1Ts.astype(ml_dtypes.bfloat16)

    # x0 pre-scaled, bf16, chunk-major [128, tiles*HID]
    x0s = np.zeros((M, p.shard_pad, p.HID), np.float32)
    for m in range(M):
        x0s[m, :p.shard] = X0_SCALE * x_0[m * p.shard:(m + 1) * p.shard]
    x0s = x0s.reshape(M, p.tiles, 128, p.HID).transpose(0, 2, 1, 3).reshape(M, 128, -1)
    x0s = np.ascontiguousarray(x0s).astype(ml_dtypes.bfloat16)

    iota = np.broadcast_to(np.arange(128, dtype=np.float32), (128, 128)).astype(ml_dtypes.bfloat16).copy()
    ident = np.eye(128, dtype=np.float32)
    ones1 = np.ones((1, 128), np.float32)
    onesc = np.ones((128, 1), np.float32)

    meta = dict(K=K, off=off, ktot=ktot, num_reg=num_reg, pairs=pairs,
                npairs=npairs, Kmax=Kmax)
    shared = dict(iota=iota, ident=ident, ones1=ones1, onesc=onesc)
    in_maps = []
    for m in range(M):
        d = dict(shared)
        d["s1T"] = np.ascontiguousarray(s1Ts[m])
        d["idxall"] = np.ascontiguousarray(idx_lay[m])
        d["drel"] = np.ascontiguousarray(drel_flat[m])
        d["x0s"] = x0s[m]
        in_maps.append(d)
    return in_maps, meta


def build_kernel(prob, meta, W_pre, gamma, beta_bn, W_op, nloop=1, nq=1,
                 phases="13C", coll=True):
    p = prob
    K, off, ktot = meta["K"], meta["off"], meta["ktot"]
    num_reg, pairs, npairs = meta["num_reg"], meta["pairs"], meta["npairs"]
    Kmax = meta["Kmax"]
    C, HID = p.C, p.HID
    M = p.n_cores
    cpr = p.tiles
    NB = p.nb
    nc = bacc.Bacc("TRN2", target_bir_lowering=False, debug=False,
                   num_devices=M, num_swdge_queues=nq)
    t_s1T = nc.dram_tensor("s1T", [C, p.shard_pad], BF16, kind="ExternalInput")
    t_wpre = nc.dram_tensor("wpre", [C, HID], F32, kind="ExternalInput")
    t_gamma = nc.dram_tensor("gamma", [1, HID], F32, kind="ExternalInput")
    t_beta = nc.dram_tensor("beta", [1, HID], F32, kind="ExternalInput")
    t_wop = nc.dram_tensor("wop", [HID, HID], F32, kind="ExternalInput")
    t_x0 = nc.dram_tensor("x0s", [128, cpr * HID], BF16, kind="ExternalInput")
    t_idx = nc.dram_tensor("idxall", [128, ktot * 8], I16, kind="ExternalInput")
    t_drel = nc.dram_tensor("drel", [128, npairs], F32, kind="ExternalInput")
    t_iota = nc.dram_tensor("iota", [128, 128], BF16, kind="ExternalInput")
    t_ident = nc.dram_tensor("ident", [128, 128], F32, kind="ExternalInput")
    t_ones1 = nc.dram_tensor("ones1", [1, 128], F32, kind="ExternalInput")
    t_onesc = nc.dram_tensor("onesc", [128, 1], F32, kind="ExternalInput")
    t_out = nc.dram_tensor("out", [128, cpr * HID], F32, kind="ExternalOutput")
    # collective buffers
    t_arin = nc.dram_tensor("arin", [1, 2 * HID], F32)
    t_arout = nc.dram_tensor("arout", [1, 2 * HID], F32)
    t_hb = nc.dram_tensor("hb", [128, cpr * HID], BF16)
    t_ag = nc.dram_tensor("agh", [M * 128, cpr * HID], BF16)
    groups = [list(range(M))]

    def loop(tc, body, active):
        if active and nloop > 1:
            with tc.For_i(0, nloop, 1):
                body()
        else:
            body()

    tc1 = TileContext(nc)
    with tc1 as tc:
        with (tc.tile_pool(name="const", bufs=1) as cpool,
              tc.tile_pool(name="work", bufs=5) as wpool,
              tc.tile_pool(name="hout", bufs=4) as hpool,
              tc.tile_pool(name="psZ", bufs=4, space="PSUM") as psZ,
              tc.tile_pool(name="psB", bufs=1, space="PSUM") as psB,
              tc.tile_pool(name="psS", bufs=1, space="PSUM") as psS,
              tc.tile_pool(name="small", bufs=1) as smpool):
            # ---- constants ----
            s1sb = []
            for r in range(2):
                t = cpool.tile([128, p.shard_pad], BF16, tag=f"s1{r}")
                nc.sync.dma_start(out=t[:], in_=t_s1T[r * 128:(r + 1) * 128, :])
                s1sb.append(t)
            w_f32, w_bf = [], []
            for r in range(2):
                w = cpool.tile([128, HID], F32, tag=f"wf{r}")
                nc.sync.dma_start(out=w[:], in_=t_wpre[r * 128:(r + 1) * 128, :])
                w_f32.append(w)
                wb = cpool.tile([128, HID], BF16, tag=f"wb{r}")
                nc.vector.tensor_copy(out=wb[:], in_=w[:])
                w_bf.append(wb)
            gamma_sb = cpool.tile([1, HID], F32, tag="gm")
            nc.sync.dma_start(out=gamma_sb[:], in_=t_gamma[:])
            beta_sb = cpool.tile([1, HID], F32, tag="bt")
            nc.sync.dma_start(out=beta_sb[:], in_=t_beta[:])
            ones1_sb = cpool.tile([1, 128], F32, tag="on")
            nc.sync.dma_start(out=ones1_sb[:], in_=t_ones1[:])
            onesc_sb = cpool.tile([128, 1], F32, tag="onc")
            nc.sync.dma_start(out=onesc_sb[:], in_=t_onesc[:])
            onesc_bf = cpool.tile([128, 1], BF16, tag="oncb")
            nc.vector.tensor_copy(out=onesc_bf[:], in_=onesc_sb[:])

            # ---- P1: z -> SBUF store; squares + column-sum matmuls ----
            sums_z = psS.tile([1, HID], F32, tag="sz", name="sums_z")
            sums_q = psS.tile([1, HID], F32, tag="sq", name="sums_q")
            zst = cpool.tile([128, cpr * HID], BF16, tag="zst")
            zqt = cpool.tile([128, cpr * HID], BF16, tag="zqt")
            def phase1():
                for j in range(cpr):
                    zc = psZ.tile([128, HID], F32, tag="zc")
                    co = j * 128
                    nc.tensor.matmul(zc[:], lhsT=s1sb[0][:, co:co + 128],
                                     rhs=w_bf[0][:], start=True, stop=False)
                    nc.tensor.matmul(zc[:], lhsT=s1sb[1][:, co:co + 128],
                                     rhs=w_bf[1][:], start=False, stop=True)
                    if j % 2 == 0:
                        nc.vector.tensor_copy(out=zst[:, j * HID:(j + 1) * HID],
                                              in_=zc[:])
                    else:
                        nc.scalar.activation(out=zst[:, j * HID:(j + 1) * HID],
                                             in_=zc[:],
                                             func=mybir.ActivationFunctionType.Copy,
                                             bias=0.0, scale=1.0)
                for j in range(cpr):
                    nc.scalar.activation(out=zqt[:, j * HID:(j + 1) * HID],
                                         in_=zst[:, j * HID:(j + 1) * HID],
                                         func=mybir.ActivationFunctionType.Square,
                                         bias=0.0, scale=1.0)
                for j in range(cpr):
                    nc.tensor.matmul(sums_z[:], lhsT=onesc_bf[:],
                                     rhs=zst[:, j * HID:(j + 1) * HID],
                                     start=(j == 0), stop=(j == cpr - 1))
                    nc.tensor.matmul(sums_q[:], lhsT=onesc_bf[:],
                                     rhs=zqt[:, j * HID:(j + 1) * HID],
                                     start=(j == 0), stop=(j == cpr - 1))
            loop(tc, phase1, "1" in phases)

            # ---- AR: allreduce stats ----
            sums_sb = smpool.tile([1, 2 * HID], F32, tag="ssb")
            nc.vector.tensor_copy(out=sums_sb[:, :HID], in_=sums_z[:])
            nc.vector.tensor_copy(out=sums_sb[:, HID:], in_=sums_q[:])
            nc.sync.dma_start(out=t_arin[:], in_=sums_sb[:])
            def phase_ar():
                nc.gpsimd.collective_compute(
                    "AllReduce", mybir.AluOpType.add, replica_groups=groups,
                    ins=[t_arin[:]], outs=[t_arout[:]])
            if coll:
                loop(tc, phase_ar, "r" in phases)
            stats_sb = smpool.tile([1, 2 * HID], F32, tag="stats")
            nc.sync.dma_start(out=stats_sb[:],
                              in_=(t_arout[:] if coll else t_arin[:]))

            # ---- stats finalize ----
            invn = 1.0 / p.N
            mu = smpool.tile([1, HID], F32, tag="mu")
            nc.vector.tensor_scalar(out=mu[:], in0=stats_sb[:, :HID], scalar1=invn,
                                    scalar2=None, op0=mybir.AluOpType.mult)
            var = smpool.tile([1, HID], F32, tag="var")
            nc.vector.tensor_scalar(out=var[:], in0=stats_sb[:, HID:], scalar1=invn,
                                    scalar2=None, op0=mybir.AluOpType.mult)
            musq = smpool.tile([1, HID], F32, tag="musq")
            nc.vector.tensor_tensor(out=musq[:], in0=mu[:], in1=mu[:],
                                    op=mybir.AluOpType.mult)
            nc.vector.tensor_tensor(out=var[:], in0=var[:], in1=musq[:],
                                    op=mybir.AluOpType.subtract)
            nc.vector.tensor_scalar(out=var[:], in0=var[:], scalar1=BN_EPS,
                                    scalar2=None, op0=mybir.AluOpType.add)
            sq = smpool.tile([1, HID], F32, tag="sqr")
            nc.scalar.activation(out=sq[:], in_=var[:],
                                 func=mybir.ActivationFunctionType.Sqrt,
                                 bias=0.0, scale=1.0)
            rs = smpool.tile([1, HID], F32, tag="rs")
            nc.vector.reciprocal(out=rs[:], in_=sq[:])
            a_vec = smpool.tile([1, HID], F32, tag="av")
            nc.vector.tensor_tensor(out=a_vec[:], in0=rs[:], in1=gamma_sb[:],
                                    op=mybir.AluOpType.mult)
            b_vec = smpool.tile([1, HID], F32, tag="bv")
            nc.vector.tensor_tensor(out=b_vec[:], in0=mu[:], in1=a_vec[:],
                                    op=mybir.AluOpType.mult)
            nc.vector.tensor_tensor(out=b_vec[:], in0=beta_sb[:], in1=b_vec[:],
                                    op=mybir.AluOpType.subtract)
            ps_ab = psB.tile([128, HID], F32, tag="zab", name="ps_ab")
            nc.tensor.matmul(ps_ab[:], lhsT=ones1_sb[:], rhs=a_vec[:],
                             start=True, stop=True)
            a_bc = cpool.tile([128, HID], F32, tag="abc2")
            nc.vector.tensor_copy(out=a_bc[:], in_=ps_ab[:])
            # fold the BN scale into the weights: W' = W * a  (bf16)
            w_sc = []
            for r in range(2):
                wpr = cpool.tile([128, HID], BF16, tag=f"wpr{r}")
                nc.vector.tensor_tensor(out=wpr[:], in0=w_f32[r][:], in1=a_bc[:],
                                        op=mybir.AluOpType.mult)
                w_sc.append(wpr)

            # ---- P3: h = relu(s1 @ W' + b) (PE + Act only) ----
            HSPAN = 7
            def phase3():
                for j in range(cpr):
                    hc = psZ.tile([128, HID], F32, tag="zc")
                    co = j * 128
                    nc.tensor.matmul(hc[:], lhsT=s1sb[0][:, co:co + 128],
                                     rhs=w_sc[0][:], start=True, stop=False)
                    nc.tensor.matmul(hc[:], lhsT=s1sb[1][:, co:co + 128],
                                     rhs=w_sc[1][:], start=False, stop=False)
                    nc.tensor.matmul(hc[:], lhsT=ones1_sb[:], rhs=b_vec[:],
                                     start=False, stop=True)
                    hs = j // HSPAN
                    ho = j % HSPAN
                    he = min(cpr, (hs + 1) * HSPAN) - hs * HSPAN
                    if ho == 0:
                        hsp = hpool.tile([128, HSPAN * HID], BF16, tag="hsp",
                                         name=f"hsp_{hs % 4}")
                        phase3.hsp = hsp
                    hsp = phase3.hsp
                    hj = hsp[:, ho * HID:(ho + 1) * HID]
                    if j % 2 == 0:
                        nc.scalar.activation(out=hj, in_=hc[:],
                                             func=mybir.ActivationFunctionType.Relu,
                                             bias=0.0, scale=1.0)
                    else:
                        nc.vector.tensor_scalar(out=hj, in0=hc[:], scalar1=0.0,
                                                scalar2=None,
                                                op0=mybir.AluOpType.max)
                    if ho == he - 1:
                        nc.sync.dma_start(
                            out=t_hb[:, hs * HSPAN * HID:(hs * HSPAN + he) * HID],
                            in_=hsp[:, :he * HID])
            loop(tc, phase3, "3" in phases)

            # ---- AG: allgather h shards ----
            def phase_ag():
                nc.gpsimd.collective_compute(
                    "AllGather", mybir.AluOpType.bypass, replica_groups=groups,
                    ins=[t_hb[:]], outs=[t_ag[:]])
            if coll:
                loop(tc, phase_ag, "G" in phases)

        # ---------------- phase C: aggregate + output ----------------
        with (tc.tile_pool(name="c2", bufs=1) as cpool,
              tc.tile_pool(name="gat", bufs=1) as gpool,
              tc.tile_pool(name="sel", bufs=16) as selp,
              tc.tile_pool(name="epi", bufs=3) as epool,
              tc.tile_pool(name="osp", bufs=2) as opool,
              tc.tile_pool(name="psG", bufs=4, space="PSUM") as psG,
              tc.tile_pool(name="psT", bufs=2, space="PSUM") as psT,
              tc.tile_pool(name="psO", bufs=2, space="PSUM") as psO):
            idx_sb = cpool.tile([128, ktot * 8], I16, tag="idx")
            nc.sync.dma_start(out=idx_sb[:], in_=t_idx[:])
            drel_sb = cpool.tile([128, npairs], F32, tag="dr")
            nc.sync.dma_start(out=drel_sb[:], in_=t_drel[:])
            iota_sb = cpool.tile([128, 128], BF16, tag="io")
            nc.sync.dma_start(out=iota_sb[:], in_=t_iota[:])
            ident_sb = cpool.tile([128, 128], F32, tag="idn")
            nc.sync.dma_start(out=ident_sb[:], in_=t_ident[:])
            ident_bf = cpool.tile([128, 128], BF16, tag="idnb")
            nc.vector.tensor_copy(out=ident_bf[:], in_=ident_sb[:])
            hsh = cpool.tile([128, cpr * HID], BF16, tag="hsh")
            nc.sync.dma_start(out=hsh[:], in_=t_hb[:])
            x0sb = cpool.tile([128, cpr * HID], BF16, tag="x0sb")
            nc.sync.dma_start(out=x0sb[:], in_=t_x0[:])
            # W_eff = I + W_OP_SCALE * W_op  (bf16, 2 chunks)
            weff = []
            for r in range(2):
                w = cpool.tile([128, HID], F32, tag=f"wo2{r}")
                nc.sync.dma_start(out=w[:], in_=t_wop[r * 128:(r + 1) * 128, :])
                wsc = cpool.tile([128, HID], F32, tag=f"wsc{r}")
                nc.vector.tensor_scalar(out=wsc[:], in0=w[:], scalar1=W_OP_SCALE,
                                        scalar2=None, op0=mybir.AluOpType.mult)
                nc.vector.tensor_tensor(
                    out=wsc[:, r * 128:(r + 1) * 128],
                    in0=wsc[:, r * 128:(r + 1) * 128], in1=ident_sb[:],
                    op=mybir.AluOpType.add)
                wb = cpool.tile([128, HID], BF16, tag=f"wo2b{r}")
                nc.vector.tensor_copy(out=wb[:], in_=wsc[:])
                weff.append(wb)

            half_parts = (M // 2) * 128
            tbl = [t_ag[0:half_parts, :], t_ag[half_parts:2 * half_parts, :]]

            # gather buffers (uniform size), memset once for -1-skip safety
            for h in (0, 1):
                for par in (0, 1):
                    g = gpool.tile([128, Kmax * HID], BF16, name=f"g{h}_{par}")
                    nc.vector.memset(g[:], 0.0)
            qn = [0]

            # per-tile matmul counts for stop flags
            nmm = [2] * cpr
            for b in range(NB):
                for (h, c, t, j) in pairs[b]:
                    nmm[t] += 1

            OSP = TB
            def phaseC():
                for b in range(NB):
                    t0 = b * TB
                    t1 = min(cpr, t0 + TB)
                    par = b % 2
                    gb = {}
                    for h in (0, 1):
                        kk = int(K[b, h])
                        g = gpool.tile([128, Kmax * HID], BF16, name=f"g{h}_{par}")
                        o8 = int(off[b, h]) * 8
                        nc.gpsimd.dma_gather(
                            out_ap=g[:, :kk * HID].rearrange("p (c d) -> p c d", d=HID),
                            in_ap=tbl[h].rearrange("q (c d) -> (q c) d", d=HID),
                            idxs_ap=idx_sb[:, o8:o8 + kk * 8],
                            num_idxs=kk * 128, num_idxs_reg=kk * 128,
                            elem_size=HID, single_packet=False,
                            queue_num=qn[0] % nq)
                        qn[0] += 1
                        gb[h] = g
                    aggs = {}
                    done = {}
                    for t in range(t0, t1):
                        agg = psG.tile([128, HID], F32, tag="agg",
                                       name=f"agg{t % 4}")
                        nc.tensor.matmul(agg[:], lhsT=ident_bf[:],
                                         rhs=hsh[:, t * HID:(t + 1) * HID],
                                         start=True, stop=False)
                        nc.tensor.matmul(agg[:], lhsT=ident_bf[:],
                                         rhs=x0sb[:, t * HID:(t + 1) * HID],
                                         start=False, stop=(nmm[t] == 2))
                        aggs[t] = agg
                        done[t] = 2
                    for (h, c, t, j) in pairs[b]:
                        S = selp.tile([128, 128], BF16)
                        nc.vector.tensor_scalar(
                            out=S[:], in0=iota_sb[:],
                            scalar1=drel_sb[:, j:j + 1], scalar2=None,
                            op0=mybir.AluOpType.is_equal)
                        done[t] += 1
                        nc.tensor.matmul(aggs[t][:], lhsT=S[:],
                                         rhs=gb[h][:, c * HID:(c + 1) * HID],
                                         start=False, stop=(done[t] == nmm[t]))
                    # epilogue for the block
                    outsp = opool.tile([128, OSP * HID], F32, tag="outsp")
                    for t in range(t0, t1):
                        eo = t - t0
                        sup = epool.tile([128, HID], F32, tag="sup")
                        nc.scalar.activation(out=sup[:], in_=aggs[t][:],
                                             func=mybir.ActivationFunctionType.Copy,
                                             bias=0.0, scale=(1.0 - ALPHA))
                        trp = psT.tile([128, HID], F32, tag="tr")
                        for r in range(2):
                            nc.tensor.transpose(
                                out=trp[:, r * 128:(r + 1) * 128],
                                in_=sup[:, r * 128:(r + 1) * 128],
                                identity=ident_sb[:])
                        supT = epool.tile([128, HID], BF16, tag="supT")
                        if t % 2 == 0:
                            nc.vector.tensor_copy(out=supT[:], in_=trp[:])
                        else:
                            nc.scalar.activation(
                                out=supT[:], in_=trp[:],
                                func=mybir.ActivationFunctionType.Copy,
                                bias=0.0, scale=1.0)
                        ops = psO.tile([128, HID], F32, tag="o")
                        nc.tensor.matmul(ops[:], lhsT=supT[:, :128], rhs=weff[0][:],
                                         start=True, stop=False)
                        nc.tensor.matmul(ops[:], lhsT=supT[:, 128:], rhs=weff[1][:],
                                         start=False, stop=True)
                        nc.scalar.activation(out=outsp[:, eo * HID:(eo + 1) * HID],
                                             in_=ops[:],
                                             func=mybir.ActivationFunctionType.Relu,
                                             bias=0.0, scale=OUT_SCALE)
                    nc.sync.dma_start(out=t_out[:, t0 * HID:t1 * HID],
                                      in_=outsp[:, :(t1 - t0) * HID])
            loop(tc, phaseC, "C" in phases)

    nc.compile()
    return nc


def make_weight_inputs(prob, W_pre, gamma, beta_bn, W_op):
    return dict(
        wpre=np.asarray(W_pre, np.float32),
        gamma=np.asarray(gamma, np.float32).reshape(1, -1),
        beta=np.asarray(beta_bn, np.float32).reshape(1, -1),
        wop=np.asarray(W_op, np.float32),
    )


def unpack_out(prob, arr):
    return arr.reshape(128, prob.tiles, prob.HID).transpose(1, 0, 2).reshape(
        prob.shard_pad, prob.HID)


# ======================================================================
# Self-contained execution via PJRT (axon)
# ======================================================================
import jax
from jax.sharding import Mesh, PartitionSpec, NamedSharding
from jax.experimental.shard_map import shard_map
from concourse.bass2jax import _bass_exec_p, install_neuronx_cc_hook, partition_id_tensor


def _build_exec(nc, n_cores):
    install_neuronx_cc_hook()
    partition_name = nc.partition_id_tensor.name if nc.partition_id_tensor else None
    in_names, out_names, out_avals, zero_outs = [], [], [], []
    for alloc in nc.m.functions[0].allocations:
        if not isinstance(alloc, mybir.MemoryLocationSet):
            continue
        name = alloc.memorylocations[0].name
        if alloc.kind == "ExternalInput":
            if name != partition_name:
                in_names.append(name)
        elif alloc.kind == "ExternalOutput":
            shape = tuple(alloc.tensor_shape)
            dtype = mybir.dt.np(alloc.dtype)
            out_names.append(name)
            out_avals.append(jax.core.ShapedArray(shape, dtype))
            zero_outs.append(np.zeros(shape, dtype))
    n_params = len(in_names)
    n_outs = len(out_avals)
    all_in_names = list(in_names) + list(out_names)
    if partition_name is not None:
        all_in_names.append(partition_name)

    def _body(*args):
        operands = list(args)
        if partition_name is not None:
            operands.append(partition_id_tensor())
        outs = _bass_exec_p.bind(
            *operands, out_avals=tuple(out_avals), in_names=tuple(all_in_names),
            out_names=tuple(out_names), lowering_input_output_aliases=(),
            sim_require_finite=True, sim_require_nnan=True, nc=nc)
        return tuple(outs)

    devices = jax.devices()[:n_cores]
    mesh = Mesh(np.asarray(devices), ("core",))
    in_specs = (PartitionSpec("core"),) * (n_params + n_outs)
    out_specs = (PartitionSpec("core"),) * n_outs
    donate = tuple(range(n_params, n_params + n_outs))
    fn = jax.jit(shard_map(_body, mesh=mesh, in_specs=in_specs,
                           out_specs=out_specs, check_rep=False),
                 donate_argnums=donate, keep_unused=True)
    return dict(fn=fn, in_names=in_names, out_names=out_names,
                out_avals=out_avals, zero_outs=zero_outs, mesh=mesh,
                n_cores=n_cores)


def _place_inputs(ex, in_maps):
    sh = NamedSharding(ex["mesh"], PartitionSpec("core"))
    n_cores = ex["n_cores"]
    return [jax.device_put(
        np.concatenate([np.asarray(in_maps[c][name]) for c in range(n_cores)], axis=0), sh)
        for name in ex["in_names"]]


def _run(ex, dev_in):
    sh = NamedSharding(ex["mesh"], PartitionSpec("core"))
    n_cores = ex["n_cores"]
    zs = [jax.device_put(np.zeros((n_cores * z.shape[0], *z.shape[1:]), z.dtype), sh)
          for z in ex["zero_outs"]]
    outs = jax.block_until_ready(ex["fn"](*dev_in, *zs))
    return [
        {name: np.asarray(outs[i]).reshape(n_cores, *ex["out_avals"][i].shape)[c]
         for i, name in enumerate(ex["out_names"])}
        for c in range(n_cores)
    ]


_CACHE = {}


def _get_compiled(prob, meta, W_pre, gamma, beta_bn, W_op, key):
    if key not in _CACHE:
        nc = build_kernel(prob, meta, W_pre, gamma, beta_bn, W_op, nloop=1)
        _CACHE[key] = _build_exec(nc, prob.n_cores)
    return _CACHE[key]


def kernel(s0=None, s1=None, x_0=None, W_pre=None, gamma=None, beta_bn=None,
           W_op=None, edge_index=None, drop_prob=None, training=None, **_ignored):
    s1 = np.asarray(s1, np.float32)
    x_0 = np.asarray(x_0, np.float32)
    W_pre = np.asarray(W_pre, np.float32)
    gamma = np.asarray(gamma, np.float32)
    beta_bn = np.asarray(beta_bn, np.float32)
    W_op = np.asarray(W_op, np.float32)
    edge_index = np.asarray(edge_index)
    N, C = s1.shape
    HID = W_pre.shape[1]
    E = edge_index.shape[1]
    prob = Prob(N, E, C, HID, n_cores=8)
    in_maps, meta = host_prep(prob, s1, x_0, edge_index)
    key = (N, E, C, HID, int(np.int64(edge_index[:, ::97]).sum()), meta["ktot"])
    ex = _get_compiled(prob, meta, W_pre, gamma, beta_bn, W_op, key)
    wins = make_weight_inputs(prob, W_pre, gamma, beta_bn, W_op)
    full_maps = [{**m, **wins} for m in in_maps]
    dev_in = _place_inputs(ex, full_maps)
    res = _run(ex, dev_in)
    out = np.concatenate(
        [unpack_out(prob, res[m]["out"])[:prob.shard] for m in range(prob.n_cores)],
        axis=0)
    return np.ascontiguousarray(out[:N]).astype(np.float32)


# revision 14
# speedup vs baseline: 1.7303x; 1.5859x over previous
"""GNN message-passing kernel v4 for trn2 (8 cores).

h = relu(BN(s1 @ W_pre)); agg = segment_sum(h[src], dst);
out = relu((1-b)*support + b*support@W_op), support = 0.9*(h+agg) + 0.1*x_0.

Sharding: s1/x_0/dst sharded by node across 8 cores.
- P1: per-core z = s1_m @ W (bf16) chunk matmuls; column sums of z and z^2
  accumulated in PSUM via ones-lhsT matmuls; z stored in SBUF (bf16).
- AR: AllReduce the [1,512] partial sums -> global BN stats; broadcast the
  affine (a, b) rows.
- P3: h_m = relu(a*z + b) -> bounce to DRAM (bf16, chunk-major).
- AG: AllGather h shards -> full gather table [1024, 49*256] in local DRAM.
- C: dst tiles grouped into blocks of TB tiles; edges packed per
  (block, table-half) with no per-tile chunk padding (chunks may straddle
  tile boundaries -> one selection matmul per (chunk, tile) pair). One
  dma_gather per (block, half) with exact runtime count; -1 padding
  indices are skipped by HW. x0 (pre-scaled bf16, SBUF-resident) and the
  self term enter the PSUM accumulation via identity matmuls. Epilogue
  applies W_eff = I + scale*W_op so no extra adds are needed.
"""
import math
import os
import numpy as np
import ml_dtypes

import concourse.bass as bass
import concourse.bacc as bacc
import concourse.mybir as mybir
from concourse.tile import TileContext

BF16 = mybir.dt.bfloat16
F32 = mybir.dt.float32
I16 = mybir.dt.int16

ALPHA = 0.1
LAMBDA = 0.5
BN_EPS = 1e-5
BETA_C = float(np.log(LAMBDA / 1.0 + 1.0))   # 0.405465
W_OP_SCALE = BETA_C / (1.0 - BETA_C)
OUT_SCALE = 1.0 - BETA_C
X0_SCALE = ALPHA / (1.0 - ALPHA)
TB = int(os.environ.get("KERNEL_TB", "4"))    # dst tiles per gather block


class Prob:
    def __init__(self, N, E, C, HID, n_cores):
        self.N, self.E, self.C, self.HID, self.n_cores = N, E, C, HID, n_cores
        assert C == 256 and HID == 256
        self.shard = N // n_cores                      # nodes per core
        assert self.shard * n_cores == N
        self.tiles = math.ceil(self.shard / 128)       # chunks per shard
        self.shard_pad = self.tiles * 128
        self.nb = math.ceil(self.tiles / TB)           # gather blocks
        self.half_rows = (n_cores // 2) * 128 * self.tiles
        assert self.half_rows < 32768


def host_prep(prob, s1, x_0, edge_index):
    """Per-core input maps + static schedule. All numpy."""
    p = prob
    N, E, M = p.N, p.E, p.n_cores
    s1 = np.asarray(s1, dtype=np.float32)
    x_0 = np.asarray(x_0, dtype=np.float32)
    src = np.asarray(edge_index[0], dtype=np.int64)
    dst = np.asarray(edge_index[1], dtype=np.int64)

    core = dst // p.shard
    rel = dst - core * p.shard
    t_of = rel >> 7                     # dst tile within shard
    prel = rel & 127                    # dst row within tile
    b_of = t_of // TB                   # gather block
    sr = src // p.shard
    sl = src - sr * p.shard
    sc = sl >> 7
    sp = sl & 127
    srow = (sr * 128 + sp) * p.tiles + sc
    half = (srow >= p.half_rows).astype(np.int64)
    srow = srow - half * p.half_rows

    NB = p.nb
    ngroups = M * NB * 2
    group = (core * NB + b_of) * 2 + half
    order = np.lexsort((srow, t_of, group))
    g_s = group[order]
    t_s = t_of[order]
    prel_s = prel[order]
    srow_s = srow[order]

    gcnt = np.bincount(group, minlength=ngroups)           # [M*NB*2]
    gstart = np.zeros(ngroups + 1, np.int64)
    np.cumsum(gcnt, out=gstart[1:])
    pos = np.arange(E, dtype=np.int64) - gstart[g_s]       # pos within group
    chunk = pos >> 7
    slot = pos & 127

    cnt_mbh = gcnt.reshape(M, NB, 2)
    num_reg = cnt_mbh.max(axis=0)                          # [NB, 2]
    K = np.maximum((num_reg + 127) // 128, 1)              # chunks per call
    off = np.zeros((NB, 2), np.int64)                      # chunk offset
    run = 0
    for b in range(NB):
        for h in (0, 1):
            off[b, h] = run
            run += K[b, h]
    ktot = run

    # ---- static (b, h, chunk, tile) pair schedule (union over cores) ----
    bh_s = g_s % (NB * 2)                                  # (b*2 + h)
    Kmax = int(K.max())
    pair_key = (bh_s * (Kmax + 1) + chunk) * 64 + (t_s - (bh_s // 2) * TB)
    upairs, inv = np.unique(pair_key, return_inverse=True)
    npairs = len(upairs)
    u_bh = upairs // ((Kmax + 1) * 64)
    u_rest = upairs % ((Kmax + 1) * 64)
    u_chunk = u_rest // 64
    u_tin = u_rest % 64
    u_b = u_bh // 2
    u_h = u_bh % 2
    u_t = u_b * TB + u_tin
    # pair order: already sorted by (b, h, chunk, tile) via key  ✓
    pairs = [[] for _ in range(NB)]     # per block: list of (h, c, t, j)
    for j in range(npairs):
        pairs[int(u_b[j])].append((int(u_h[j]), int(u_chunk[j]), int(u_t[j]), j))

    # ---- per-core flat layouts ----
    edge_core = g_s // (NB * 2)
    flatpos = (off[u_b[inv], u_h[inv]] * 128 + pos)        # idx slot per edge
    idx_flat = np.zeros((M, ktot * 128), np.int16)
    idx_flat[edge_core, flatpos] = srow_s.astype(np.int16)
    drel_flat = np.full((M, 128, npairs), 200.0, np.float32)
    drel_flat[edge_core, slot, inv] = prel_s.astype(np.float32)

    idx_lay = idx_flat.reshape(M, ktot * 8, 16).transpose(0, 2, 1)
    idx_lay = np.tile(idx_lay, (1, 8, 1))                  # [M, 128, ktot*8]

    # per-core s1 shard, transposed+padded: [C, shard_pad] bf16
    s1Ts = np.zeros((M, p.C, p.shard_pad), np.float32)
    for m in range(M):
        s1Ts[m, :, :p.shard] = s1[m * p.shard:(m + 1) * p.shard].T
    s1Ts = s1Ts.astype(ml_dtypes.bfloat16)

    # x0 pre-scaled, bf16, chunk-major [128, tiles*HID]
    x0s = np.zeros((M, p.shard_pad, p.HID), np.float32)
    for m in range(M):
        x0s[m, :p.shard] = X0_SCALE * x_0[m * p.shard:(m + 1) * p.shard]
    x0s = x0s.reshape(M, p.tiles, 128, p.HID).transpose(0, 2, 1, 3).reshape(M, 128, -1)
    x0s = np.ascontiguousarray(x0s).astype(ml_dtypes.bfloat16)

    iota = np.broadcast_to(np.arange(128, dtype=np.float32), (128, 128)).astype(ml_dtypes.bfloat16).copy()
    ident = np.eye(128, dtype=np.float32)
    ones1 = np.ones((1, 128), np.float32)
    onesc = np.ones((128, 1), np.float32)

    meta = dict(K=K, off=off, ktot=ktot, num_reg=num_reg, pairs=pairs,
                npairs=npairs, Kmax=Kmax)
    shared = dict(iota=iota, ident=ident, ones1=ones1, onesc=onesc)
    in_maps = []
    for m in range(M):
        d = dict(shared)
        d["s1T"] = np.ascontiguousarray(s1Ts[m])
        d["idxall"] = np.ascontiguousarray(idx_lay[m])
        d["drel"] = np.ascontiguousarray(drel_flat[m])
        d["x0s"] = x0s[m]
        in_maps.append(d)
    return in_maps, meta


def build_kernel(prob, meta, W_pre, gamma, beta_bn, W_op, nloop=1, nq=1,
                 phases="13C", coll=True):
    p = prob
    K, off, ktot = meta["K"], meta["off"], meta["ktot"]
    num_reg, pairs, npairs = meta["num_reg"], meta["pairs"], meta["npairs"]
    Kmax = meta["Kmax"]
    C, HID = p.C, p.HID
    M = p.n_cores
    cpr = p.tiles
    NB = p.nb
    nc = bacc.Bacc("TRN2", target_bir_lowering=False, debug=False,
                   num_devices=M, num_swdge_queues=nq)
    t_s1T = nc.dram_tensor("s1T", [C, p.shard_pad], BF16, kind="ExternalInput")
    t_wpre = nc.dram_tensor("wpre", [C, HID], F32, kind="ExternalInput")
    t_gamma = nc.dram_tensor("gamma", [1, HID], F32, kind="ExternalInput")
    t_beta = nc.dram_tensor("beta", [1, HID], F32, kind="ExternalInput")
    t_wop = nc.dram_tensor("wop", [HID, HID], F32, kind="ExternalInput")
    t_x0 = nc.dram_tensor("x0s", [128, cpr * HID], BF16, kind="ExternalInput")
    t_idx = nc.dram_tensor("idxall", [128, ktot * 8], I16, kind="ExternalInput")
    t_drel = nc.dram_tensor("drel", [128, npairs], F32, kind="ExternalInput")
    t_iota = nc.dram_tensor("iota", [128, 128], BF16, kind="ExternalInput")
    t_ident = nc.dram_tensor("ident", [128, 128], F32, kind="ExternalInput")
    t_ones1 = nc.dram_tensor("ones1", [1, 128], F32, kind="ExternalInput")
    t_onesc = nc.dram_tensor("onesc", [128, 1], F32, kind="ExternalInput")
    t_out = nc.dram_tensor("out", [128, cpr * HID], F32, kind="ExternalOutput")
    # collective buffers
    t_arin = nc.dram_tensor("arin", [1, 2 * HID], F32)
    t_arout = nc.dram_tensor("arout", [1, 2 * HID], F32)
    t_hb = nc.dram_tensor("hb", [128, cpr * HID], BF16)
    t_ag = nc.dram_tensor("agh", [M * 128, cpr * HID], BF16)
    groups = [list(range(M))]

    def loop(tc, body, active):
        if active and nloop > 1:
            with tc.For_i(0, nloop, 1):
                body()
        else:
            body()

    tc1 = TileContext(nc)
    with tc1 as tc:
        with (tc.tile_pool(name="const", bufs=1) as cpool,
              tc.tile_pool(name="work", bufs=5) as wpool,
              tc.tile_pool(name="hout", bufs=4) as hpool,
              tc.tile_pool(name="psZ", bufs=4, space="PSUM") as psZ,
              tc.tile_pool(name="psB", bufs=1, space="PSUM") as psB,
              tc.tile_pool(name="psS", bufs=1, space="PSUM") as psS,
              tc.tile_pool(name="small", bufs=1) as smpool):
            # ---- constants ----
            s1sb = []
            for r in range(2):
                t = cpool.tile([128, p.shard_pad], BF16, tag=f"s1{r}")
                nc.sync.dma_start(out=t[:], in_=t_s1T[r * 128:(r + 1) * 128, :])
                s1sb.append(t)
            w_f32, w_bf = [], []
            for r in range(2):
                w = cpool.tile([128, HID], F32, tag=f"wf{r}")
                nc.sync.dma_start(out=w[:], in_=t_wpre[r * 128:(r + 1) * 128, :])
                w_f32.append(w)
                wb = cpool.tile([128, HID], BF16, tag=f"wb{r}")
                nc.vector.tensor_copy(out=wb[:], in_=w[:])
                w_bf.append(wb)
            gamma_sb = cpool.tile([1, HID], F32, tag="gm")
            nc.sync.dma_start(out=gamma_sb[:], in_=t_gamma[:])
            beta_sb = cpool.tile([1, HID], F32, tag="bt")
            nc.sync.dma_start(out=beta_sb[:], in_=t_beta[:])
            ones1_sb = cpool.tile([1, 128], F32, tag="on")
            nc.sync.dma_start(out=ones1_sb[:], in_=t_ones1[:])
            onesc_sb = cpool.tile([128, 1], F32, tag="onc")
            nc.sync.dma_start(out=onesc_sb[:], in_=t_onesc[:])
            onesc_bf = cpool.tile([128, 1], BF16, tag="oncb")
            nc.vector.tensor_copy(out=onesc_bf[:], in_=onesc_sb[:])

            # ---- P1: z -> SBUF store; squares + column-sum matmuls ----
            sums_z = psS.tile([1, HID], F32, tag="sz", name="sums_z")
            sums_q = psS.tile([1, HID], F32, tag="sq", name="sums_q")
            zst = cpool.tile([128, cpr * HID], BF16, tag="zst")
            zqt = cpool.tile([128, cpr * HID], BF16, tag="zqt")
            def phase1():
                for j in range(cpr):
                    zc = psZ.tile([128, HID], F32, tag="zc")
                    co = j * 128
                    nc.tensor.matmul(zc[:], lhsT=s1sb[0][:, co:co + 128],
                                     rhs=w_bf[0][:], start=True, stop=False)
                    nc.tensor.matmul(zc[:], lhsT=s1sb[1][:, co:co + 128],
                                     rhs=w_bf[1][:], start=False, stop=True)
                    if j % 2 == 0:
                        nc.vector.tensor_copy(out=zst[:, j * HID:(j + 1) * HID],
                                              in_=zc[:])
                    else:
                        nc.scalar.activation(out=zst[:, j * HID:(j + 1) * HID],
                                             in_=zc[:],
                                             func=mybir.ActivationFunctionType.Copy,
                                             bias=0.0, scale=1.0)
                for j in range(cpr):
                    nc.scalar.activation(out=zqt[:, j * HID:(j + 1) * HID],
                                         in_=zst[:, j * HID:(j + 1) * HID],
                                         func=mybir.ActivationFunctionType.Square,
                                         bias=0.0, scale=1.0)
                for j in range(cpr):
                    nc.tensor.matmul(sums_z[:], lhsT=onesc_bf[:],
                                     rhs=zst[:, j * HID:(j + 1) * HID],
                                     start=(j == 0), stop=(j == cpr - 1))
                    nc.tensor.matmul(sums_q[:], lhsT=onesc_bf[:],
                                     rhs=zqt[:, j * HID:(j + 1) * HID],
                                     start=(j == 0), stop=(j == cpr - 1))
            loop(tc, phase1, "1" in phases)

            # ---- AR: allreduce stats ----
            sums_sb = smpool.tile([1, 2 * HID], F32, tag="ssb")
            nc.vector.tensor_copy(out=sums_sb[:, :HID], in_=sums_z[:])
            nc.vector.tensor_copy(out=sums_sb[:, HID:], in_=sums_q[:])
            nc.sync.dma_start(out=t_arin[:], in_=sums_sb[:])
            def phase_ar():
                nc.gpsimd.collective_compute(
                    "AllReduce", mybir.AluOpType.add, replica_groups=groups,
                    ins=[t_arin[:]], outs=[t_arout[:]])
            if coll:
                loop(tc, phase_ar, "r" in phases)
            stats_sb = smpool.tile([1, 2 * HID], F32, tag="stats")
            nc.sync.dma_start(out=stats_sb[:],
                              in_=(t_arout[:] if coll else t_arin[:]))

            # ---- stats finalize ----
            invn = 1.0 / p.N
            mu = smpool.tile([1, HID], F32, tag="mu")
            nc.vector.tensor_scalar(out=mu[:], in0=stats_sb[:, :HID], scalar1=invn,
                                    scalar2=None, op0=mybir.AluOpType.mult)
            var = smpool.tile([1, HID], F32, tag="var")
            nc.vector.tensor_scalar(out=var[:], in0=stats_sb[:, HID:], scalar1=invn,
                                    scalar2=None, op0=mybir.AluOpType.mult)
            musq = smpool.tile([1, HID], F32, tag="musq")
            nc.vector.tensor_tensor(out=musq[:], in0=mu[:], in1=mu[:],
                                    op=mybir.AluOpType.mult)
            nc.vector.tensor_tensor(out=var[:], in0=var[:], in1=musq[:],
                                    op=mybir.AluOpType.subtract)
            nc.vector.tensor_scalar(out=var[:], in0=var[:], scalar1=BN_EPS,
                                    scalar2=None, op0=mybir.AluOpType.add)
            sq = smpool.tile([1, HID], F32, tag="sqr")
            nc.scalar.activation(out=sq[:], in_=var[:],
                                 func=mybir.ActivationFunctionType.Sqrt,
                                 bias=0.0, scale=1.0)
            rs = smpool.tile([1, HID], F32, tag="rs")
            nc.vector.reciprocal(out=rs[:], in_=sq[:])
            a_vec = smpool.tile([1, HID], F32, tag="av")
            nc.vector.tensor_tensor(out=a_vec[:], in0=rs[:], in1=gamma_sb[:],
                                    op=mybir.AluOpType.mult)
            b_vec = smpool.tile([1, HID], F32, tag="bv")
            nc.vector.tensor_tensor(out=b_vec[:], in0=mu[:], in1=a_vec[:],
                                    op=mybir.AluOpType.mult)
            nc.vector.tensor_tensor(out=b_vec[:], in0=beta_sb[:], in1=b_vec[:],
                                    op=mybir.AluOpType.subtract)
            ps_ab = psB.tile([128, HID], F32, tag="zab", name="ps_ab")
            nc.tensor.matmul(ps_ab[:], lhsT=ones1_sb[:], rhs=a_vec[:],
                             start=True, stop=True)
            a_bc = cpool.tile([128, HID], F32, tag="abc2")
            nc.vector.tensor_copy(out=a_bc[:], in_=ps_ab[:])
            # fold the BN scale into the weights: W' = W * a  (bf16)
            w_sc = []
            for r in range(2):
                wpr = cpool.tile([128, HID], BF16, tag=f"wpr{r}")
                nc.vector.tensor_tensor(out=wpr[:], in0=w_f32[r][:], in1=a_bc[:],
                                        op=mybir.AluOpType.mult)
                w_sc.append(wpr)

            # ---- P3: h = relu(s1 @ W' + b) (PE + Act only) ----
            HSPAN = 7
            def phase3():
                for j in range(cpr):
                    hc = psZ.tile([128, HID], F32, tag="zc")
                    co = j * 128
                    nc.tensor.matmul(hc[:], lhsT=s1sb[0][:, co:co + 128],
                                     rhs=w_sc[0][:], start=True, stop=False)
                    nc.tensor.matmul(hc[:], lhsT=s1sb[1][:, co:co + 128],
                                     rhs=w_sc[1][:], start=False, stop=False)
                    nc.tensor.matmul(hc[:], lhsT=ones1_sb[:], rhs=b_vec[:],
                                     start=False, stop=True)
                    hs = j // HSPAN
                    ho = j % HSPAN
                    he = min(cpr, (hs + 1) * HSPAN) - hs * HSPAN
                    if ho == 0:
                        hsp = hpool.tile([128, HSPAN * HID], BF16, tag="hsp",
                                         name=f"hsp_{hs % 4}")
                        phase3.hsp = hsp
                    hsp = phase3.hsp
                    hj = hsp[:, ho * HID:(ho + 1) * HID]
                    if j % 2 == 0:
                        nc.scalar.activation(out=hj, in_=hc[:],
                                             func=mybir.ActivationFunctionType.Relu,
                                             bias=0.0, scale=1.0)
                    else:
                        nc.vector.tensor_scalar(out=hj, in0=hc[:], scalar1=0.0,
                                                scalar2=None,
                                                op0=mybir.AluOpType.max)
                    if ho == he - 1:
                        nc.sync.dma_start(
                            out=t_hb[:, hs * HSPAN * HID:(hs * HSPAN + he) * HID],
                            in_=hsp[:, :he * HID])
            loop(tc, phase3, "3" in phases)

            # ---- AG: allgather h shards ----
            def phase_ag():
                nc.gpsimd.collective_compute(
                    "AllGather", mybir.AluOpType.bypass, replica_groups=groups,
                    ins=[t_hb[:]], outs=[t_ag[:]])
            if coll:
                loop(tc, phase_ag, "G" in phases)

        # ---------------- phase C: aggregate + output ----------------
        with (tc.tile_pool(name="c2", bufs=1) as cpool,
              tc.tile_pool(name="gat", bufs=1) as gpool,
              tc.tile_pool(name="sel", bufs=16) as selp,
              tc.tile_pool(name="epi", bufs=3) as epool,
              tc.tile_pool(name="osp", bufs=2) as opool,
              tc.tile_pool(name="psG", bufs=4, space="PSUM") as psG,
              tc.tile_pool(name="psT", bufs=2, space="PSUM") as psT,
              tc.tile_pool(name="psO", bufs=2, space="PSUM") as psO):
            idx_sb = cpool.tile([128, ktot * 8], I16, tag="idx")
            nc.sync.dma_start(out=idx_sb[:], in_=t_idx[:])
            drel_sb = cpool.tile([128, npairs], F32, tag="dr")
            nc.sync.dma_start(out=drel_sb[:], in_=t_drel[:])
            iota_sb = cpool.tile([128, 128], BF16, tag="io")
            nc.sync.dma_start(out=iota_sb[:], in_=t_iota[:])
            ident_sb = cpool.tile([128, 128], F32, tag="idn")
            nc.sync.dma_start(out=ident_sb[:], in_=t_ident[:])
            ident_bf = cpool.tile([128, 128], BF16, tag="idnb")
            nc.vector.tensor_copy(out=ident_bf[:], in_=ident_sb[:])
            hsh = cpool.tile([128, cpr * HID], BF16, tag="hsh")
            nc.sync.dma_start(out=hsh[:], in_=t_hb[:])
            x0sb = cpool.tile([128, cpr * HID], BF16, tag="x0sb")
            nc.sync.dma_start(out=x0sb[:], in_=t_x0[:])
            # W_eff = I + W_OP_SCALE * W_op  (bf16, 2 chunks)
            weff = []
            for r in range(2):
                w = cpool.tile([128, HID], F32, tag=f"wo2{r}")
                nc.sync.dma_start(out=w[:], in_=t_wop[r * 128:(r + 1) * 128, :])
                wsc = cpool.tile([128, HID], F32, tag=f"wsc{r}")
                nc.vector.tensor_scalar(out=wsc[:], in0=w[:], scalar1=W_OP_SCALE,
                                        scalar2=None, op0=mybir.AluOpType.mult)
                nc.vector.tensor_tensor(
                    out=wsc[:, r * 128:(r + 1) * 128],
                    in0=wsc[:, r * 128:(r + 1) * 128], in1=ident_sb[:],
                    op=mybir.AluOpType.add)
                wb = cpool.tile([128, HID], BF16, tag=f"wo2b{r}")
                nc.vector.tensor_copy(out=wb[:], in_=wsc[:])
                weff.append(wb)

            half_parts = (M // 2) * 128
            tbl = [t_ag[0:half_parts, :], t_ag[half_parts:2 * half_parts, :]]

            # gather buffers (uniform size), memset once for -1-skip safety
            for h in (0, 1):
                for par in (0, 1):
                    g = gpool.tile([128, Kmax * HID], BF16, name=f"g{h}_{par}")
                    nc.vector.memset(g[:], 0.0)
            qn = [0]

            # per-tile matmul counts for stop flags
            nmm = [2] * cpr
            for b in range(NB):
                for (h, c, t, j) in pairs[b]:
                    nmm[t] += 1

            OSP = TB
            def phaseC():
                for b in range(NB):
                    t0 = b * TB
                    t1 = min(cpr, t0 + TB)
                    par = b % 2
                    gb = {}
                    for h in (0, 1):
                        kk = int(K[b, h])
                        g = gpool.tile([128, Kmax * HID], BF16, name=f"g{h}_{par}")
                        o8 = int(off[b, h]) * 8
                        nc.gpsimd.dma_gather(
                            out_ap=g[:, :kk * HID].rearrange("p (c d) -> p c d", d=HID),
                            in_ap=tbl[h].rearrange("q (c d) -> (q c) d", d=HID),
                            idxs_ap=idx_sb[:, o8:o8 + kk * 8],
                            num_idxs=kk * 128, num_idxs_reg=kk * 128,
                            elem_size=HID, single_packet=False,
                            queue_num=qn[0] % nq)
                        qn[0] += 1
                        gb[h] = g
                    aggs = {}
                    done = {}
                    for t in range(t0, t1):
                        agg = psG.tile([128, HID], F32, tag="agg",
                                       name=f"agg{t % 4}")
                        nc.tensor.matmul(agg[:], lhsT=ident_bf[:],
                                         rhs=hsh[:, t * HID:(t + 1) * HID],
                                         start=True, stop=False)
                        nc.tensor.matmul(agg[:], lhsT=ident_bf[:],
                                         rhs=x0sb[:, t * HID:(t + 1) * HID],
                                         start=False, stop=(nmm[t] == 2))
                        aggs[t] = agg
                        done[t] = 2
                    for (h, c, t, j) in pairs[b]:
                        S = selp.tile([128, 128], BF16)
                        nc.vector.tensor_scalar(
                            out=S[:], in0=iota_sb[:],
                            scalar1=drel_sb[:, j:j + 1], scalar2=None,
                            op0=mybir.AluOpType.is_equal)
                        done[t] += 1
                        nc.tensor.matmul(aggs[t][:], lhsT=S[:],
                                         rhs=gb[h][:, c * HID:(c + 1) * HID],
                                         start=False, stop=(done[t] == nmm[t]))
                    # epilogue for the block
                    outsp = opool.tile([128, OSP * HID], F32, tag="outsp")
                    for t in range(t0, t1):
                        eo = t - t0
                        sup = epool.tile([128, HID], F32, tag="sup")
                        nc.scalar.activation(out=sup[:], in_=aggs[t][:],
                                             func=mybir.ActivationFunctionType.Copy,
                                             bias=0.0, scale=(1.0 - ALPHA))
                        trp = psT.tile([128, HID], F32, tag="tr")
                        for r in range(2):
                            nc.tensor.transpose(
                                out=trp[:, r * 128:(r + 1) * 128],
                                in_=sup[:, r * 128:(r + 1) * 128],
                                identity=ident_sb[:])
                        supT = epool.tile([128, HID], BF16, tag="supT")
                        if t % 2 == 0:
                            nc.vector.tensor_copy(out=supT[:], in_=trp[:])
                        else:
                            nc.scalar.activation(
                                out=supT[:], in_=trp[:],
                                func=mybir.ActivationFunctionType.Copy,
                                bias=0.0, scale=1.0)
                        ops = psO.tile([128, HID], F32, tag="o")
                        nc.tensor.matmul(ops[:], lhsT=supT[:, :128], rhs=weff[0][:],
                                         start=True, stop=False)
                        nc.tensor.matmul(ops[:], lhsT=supT[:, 128:], rhs=weff[1][:],
                                         start=False, stop=True)
                        nc.scalar.activation(out=outsp[:, eo * HID:(eo + 1) * HID],
                                             in_=ops[:],
                                             func=mybir.ActivationFunctionType.Relu,
                                             bias=0.0, scale=OUT_SCALE)
                    nc.sync.dma_start(out=t_out[:, t0 * HID:t1 * HID],
                                      in_=outsp[:, :(t1 - t0) * HID])
            loop(tc, phaseC, "C" in phases)

    nc.compile()
    return nc


def make_weight_inputs(prob, W_pre, gamma, beta_bn, W_op):
    return dict(
        wpre=np.asarray(W_pre, np.float32),
        gamma=np.asarray(gamma, np.float32).reshape(1, -1),
        beta=np.asarray(beta_bn, np.float32).reshape(1, -1),
        wop=np.asarray(W_op, np.float32),
    )


def unpack_out(prob, arr):
    return arr.reshape(128, prob.tiles, prob.HID).transpose(1, 0, 2).reshape(
        prob.shard_pad, prob.HID)


# ======================================================================
# Self-contained execution via PJRT (axon)
# ======================================================================
import jax
from jax.sharding import Mesh, PartitionSpec, NamedSharding
from jax.experimental.shard_map import shard_map
from concourse.bass2jax import _bass_exec_p, install_neuronx_cc_hook, partition_id_tensor


def _build_exec(nc, n_cores):
    install_neuronx_cc_hook()
    partition_name = nc.partition_id_tensor.name if nc.partition_id_tensor else None
    in_names, out_names, out_avals, zero_outs = [], [], [], []
    for alloc in nc.m.functions[0].allocations:
        if not isinstance(alloc, mybir.MemoryLocationSet):
            continue
        name = alloc.memorylocations[0].name
        if alloc.kind == "ExternalInput":
            if name != partition_name:
                in_names.append(name)
        elif alloc.kind == "ExternalOutput":
            shape = tuple(alloc.tensor_shape)
            dtype = mybir.dt.np(alloc.dtype)
            out_names.append(name)
            out_avals.append(jax.core.ShapedArray(shape, dtype))
            zero_outs.append(np.zeros(shape, dtype))
    n_params = len(in_names)
    n_outs = len(out_avals)
    all_in_names = list(in_names) + list(out_names)
    if partition_name is not None:
        all_in_names.append(partition_name)

    def _body(*args):
        operands = list(args)
        if partition_name is not None:
            operands.append(partition_id_tensor())
        outs = _bass_exec_p.bind(
            *operands, out_avals=tuple(out_avals), in_names=tuple(all_in_names),
            out_names=tuple(out_names), lowering_input_output_aliases=(),
            sim_require_finite=True, sim_require_nnan=True, nc=nc)
        return tuple(outs)

    devices = jax.devices()[:n_cores]
    mesh = Mesh(np.asarray(devices), ("core",))
    in_specs = (PartitionSpec("core"),) * (n_params + n_outs)
    out_specs = (PartitionSpec("core"),) * n_outs
    donate = tuple(range(n_params, n_params + n_outs))
    fn = jax.jit(shard_map(_body, mesh=mesh, in_specs=in_specs,
                           out_specs=out_specs, check_rep=False),
                 donate_argnums=donate, keep_unused=True)
    return dict(fn=fn, in_names=in_names, out_names=out_names,
                out_avals=out_avals, zero_outs=zero_outs, mesh=mesh,
                n_cores=n_cores)


def _place_inputs(ex, in_maps):
    sh = NamedSharding(ex["mesh"], PartitionSpec("core"))
    n_cores = ex["n_cores"]
    return [jax.device_put(
        np.concatenate([np.asarray(in_maps[c][name]) for c in range(n_cores)], axis=0), sh)
        for name in ex["in_names"]]


def _run(ex, dev_in):
    sh = NamedSharding(ex["mesh"], PartitionSpec("core"))
    n_cores = ex["n_cores"]
    zs = [jax.device_put(np.zeros((n_cores * z.shape[0], *z.shape[1:]), z.dtype), sh)
          for z in ex["zero_outs"]]
    outs = jax.block_until_ready(ex["fn"](*dev_in, *zs))
    return [
        {name: np.asarray(outs[i]).reshape(n_cores, *ex["out_avals"][i].shape)[c]
         for i, name in enumerate(ex["out_names"])}
        for c in range(n_cores)
    ]


_CACHE = {}


def _get_compiled(prob, meta, W_pre, gamma, beta_bn, W_op, key):
    if key not in _CACHE:
        nc = build_kernel(prob, meta, W_pre, gamma, beta_bn, W_op, nloop=1)
        _CACHE[key] = _build_exec(nc, prob.n_cores)
    return _CACHE[key]


def kernel(s0=None, s1=None, x_0=None, W_pre=None, gamma=None, beta_bn=None,
           W_op=None, edge_index=None, drop_prob=None, training=None, **_ignored):
    s1 = np.asarray(s1, np.float32)
    x_0 = np.asarray(x_0, np.float32)
    W_pre = np.asarray(W_pre, np.float32)
    gamma = np.asarray(gamma, np.float32)
    beta_bn = np.asarray(beta_bn, np.float32)
    W_op = np.asarray(W_op, np.float32)
    edge_index = np.asarray(edge_index)
    N, C = s1.shape
    HID = W_pre.shape[1]
    E = edge_index.shape[1]
    prob = Prob(N, E, C, HID, n_cores=8)
    in_maps, meta = host_prep(prob, s1, x_0, edge_index)
    key = (N, E, C, HID, int(np.int64(edge_index[:, ::97]).sum()), meta["ktot"])
    ex = _get_compiled(prob, meta, W_pre, gamma, beta_bn, W_op, key)
    wins = make_weight_inputs(prob, W_pre, gamma, beta_bn, W_op)
    full_maps = [{**m, **wins} for m in in_maps]
    dev_in = _place_inputs(ex, full_maps)
    res = _run(ex, dev_in)
    out = np.concatenate(
        [unpack_out(prob, res[m]["out"])[:prob.shard] for m in range(prob.n_cores)],
        axis=0)
    return np.ascontiguousarray(out[:N]).astype(np.float32)
